# revision 24
# baseline (speedup 1.0000x reference)
"""DeepSets ensemble (segment mean-pool + BN MLP) on 8 TRN2 NeuronCores.

Strategy (data-parallel, per sharding hint):
 - events split 1024/core; each core's points bin-packed (FFD) into 512-pt
   groups of whole events (zero-padded); groups paired A/B so phi runs on
   [32,512] bf16 tiles with 128 output partitions (2x64 feats).
 - per j (1024 pts): phi1 matmul (w1s stationary), relu-evict h1 (DVE/ACT
   alternating), phi2 as 4 k-tile matmuls with h1 stationary producing
   h2^T (point-pair major), relu-evict h2, pooling as one 16-col matmul
   per k-tile (h2^T stationary, S moving) accumulating into a PSUM bank
   that holds ZBLK j's, with A slots in the left half-bank and B slots in
   the right half (strided matmul output AP).
 - S carries 1/len instead of 0/1 so pooling directly yields the mean;
   the bank evicts with one contiguous copy and v1 = W1 @ z runs as three
   accumulated matmuls (W1A/W1B with zeroed halves kill the junk
   quadrants, plus the x_scalar term); per-block BN statistics run inside
   the loop so only an AllGather remains at the end.
 - BatchNorm stats via PE-transposed [2,128] tensors + AllGather + on-core
   fold; a tiny warm-up AllGather at kernel start absorbs the collective
   cold-start/skew; empty-slot contributions corrected analytically;
   MLP in bf16; sigmoid + final bias applied on host.
"""
import sys
import numpy as np
import ml_dtypes
from contextlib import ExitStack

sys.path.insert(0, "/opt/trn_rl_repo")

import os
import concourse.bacc as bacc


def _FLAG(name):
    return os.environ.get(name, "1") == "1"

import concourse.tile as tile
from concourse import mybir
from concourse import bass_utils

BF16 = mybir.dt.bfloat16
F32 = mybir.dt.float32
AX = mybir.AxisListType
OP = mybir.AluOpType
ACTF = mybir.ActivationFunctionType

N_CORES = 8
C_IN = 16
F = 64
S_SCALAR = 8
M1, M2 = 128, 64
G = 512
SLOTS = 8
EPS = 1e-5

XCHUNK = 8
SCHUNK = 8
ZBLK = 32          # j per pooled-psum bank (32*16 = 512 cols)


def _plan_groups(lengths, b_total):
    e_per_core = b_total // N_CORES
    cores = []
    for c in range(N_CORES):
        evs = sorted(range(c * e_per_core, (c + 1) * e_per_core),
                     key=lambda e: -int(lengths[e]))
        groups, space = [], []
        for e in evs:
            l = int(lengths[e])
            assert 0 < l <= G
            placed = False
            for gi in range(len(groups)):
                if space[gi] >= l and len(groups[gi]) < SLOTS:
                    groups[gi].append(e)
                    space[gi] -= l
                    placed = True
                    break
            if not placed:
                groups.append([e])
                space.append(G - l)
        cores.append(groups)
    ng = max(len(g) for g in cores)
    if ng % 8:
        ng += 8 - ng % 8
    for g in cores:
        while len(g) < ng:
            g.append([])
    return cores, ng


def _slot_col(j, s, nchunk):
    """Column of slot (chunk j, slot s<16) in the z layout: per ZBLK block,
    A slots (s<8) fill the left half, B slots the right half."""
    b, jj = divmod(j, ZBLK)
    cib = min(ZBLK, nchunk - b * ZBLK)          # chunks in this block
    half = 0 if s < 8 else 1
    return b * ZBLK * 16 + half * 8 * cib + jj * 8 + (s % 8)


def _prep_core(x, x_scalar, lengths, offsets, groups, ng):
    nchunk = ng // 2
    sl = nchunk * 16
    xb = np.zeros((2 * C_IN, nchunk * G), dtype=np.float32)
    s_mat = np.zeros((128, nchunk * 64), dtype=np.float32)
    xsT = np.zeros((S_SCALAR, sl), dtype=np.float32)
    slot_events = np.full(sl, -1, dtype=np.int64)
    for j in range(nchunk):
        for half, g_idx in ((0, j), (1, nchunk + j)):
            evs = groups[g_idx]
            col0 = G * j
            row0 = C_IN * half
            pt = 0
            for i, e in enumerate(evs):
                l = int(lengths[e])
                o = int(offsets[e])
                xb[row0:row0 + C_IN, col0 + pt: col0 + pt + l] = x[:, o:o + l]
                # S: point-pair rows within k-tile; slot cols (A:0-7, B:8-15)
                # value 1/len so pooling directly produces the mean
                p_arr = np.arange(pt, pt + l)
                s = i + 8 * half
                s_mat[p_arr % 128, j * 64 + (p_arr // 128) * 16 + s] = 1.0 / l
                slot = _slot_col(j, s, nchunk)
                xsT[:, slot] = x_scalar[e]
                slot_events[slot] = e
                pt += l
    return {
        "xb": np.ascontiguousarray(xb.astype(ml_dtypes.bfloat16)),
        "S": np.ascontiguousarray(s_mat.astype(ml_dtypes.bfloat16)),
        "xsT": np.ascontiguousarray(xsT.astype(ml_dtypes.bfloat16)),
        "slot_events": slot_events,
    }


def _build_nc(ng, sl, b_total):
    nchunk = ng // 2
    n_empty = float(N_CORES * sl - b_total)
    inv_b = 1.0 / float(b_total)

    nc = bacc.Bacc("TRN2", target_bir_lowering=False, debug=False,
                   num_devices=N_CORES)

    def din(name, shape, dt):
        return nc.dram_tensor(name, shape, dt, kind="ExternalInput").ap()

    xb = din("xb", [2 * C_IN, nchunk * G], BF16)
    s_in = din("S", [128, nchunk * 64], BF16)
    xsT = din("xsT", [S_SCALAR, sl], BF16)
    w1s = din("w1s", [2 * C_IN, 128], BF16)
    w2s = din("w2s", [128, 128], BF16)
    identf = din("identf", [128, 128], F32)
    w1a = din("w1a", [128, M1], BF16)       # rows 0:64 = mw1[:, :64].T
    w1b = din("w1b", [128, M1], BF16)       # rows 64:128 = mw1[:, :64].T
    w1x = din("w1x", [S_SCALAR, M1], BF16)  # mw1[:, 64:72].T
    mw2t = din("mw2t", [M1, M2], BF16)
    mw3t = din("mw3t", [M2, 64], BF16)  # per-block slices: slice b holds w3
                                        # in col b so block b lands on y row b
    bn1_g = din("bn1_g", [M1, 1], F32)
    bn1_b = din("bn1_b", [M1, 1], F32)
    bn2_g = din("bn2_g", [M2, 1], F32)
    bn2_b = din("bn2_b", [M2, 1], F32)

    nzb = (nchunk + ZBLK - 1) // ZBLK   # number of z blocks (512 cols each)
    assert nzb <= 8
    nb1 = nzb - 1                        # blocks covered by the early gather

    y_out = nc.dram_tensor("y", [8, 512], F32, kind="ExternalOutput").ap()

    with tile.TileContext(nc) as tc, ExitStack() as ctx:
        const_pool = ctx.enter_context(tc.tile_pool(name="const", bufs=1))
        xb_pool = ctx.enter_context(tc.tile_pool(name="xb", bufs=2))
        s_pool = ctx.enter_context(tc.tile_pool(name="spool", bufs=2))
        h1_pool = ctx.enter_context(tc.tile_pool(name="h1", bufs=4))
        h2_pool = ctx.enter_context(tc.tile_pool(name="h2", bufs=4))
        z_pool = ctx.enter_context(tc.tile_pool(name="z", bufs=1))
        mlp_pool = ctx.enter_context(tc.tile_pool(name="mlp", bufs=1))
        stat_pool = ctx.enter_context(tc.tile_pool(name="stat", bufs=1))
        ps_a = ctx.enter_context(tc.tile_pool(name="psa", bufs=3, space="PSUM"))
        ps_b = ctx.enter_context(tc.tile_pool(name="psb", bufs=3, space="PSUM"))
        ps_z = ctx.enter_context(tc.tile_pool(name="psz", bufs=2, space="PSUM"))
        dram = ctx.enter_context(tc.tile_pool(name="dram", bufs=1, space="DRAM"))

        # --- activation-table preloads (Relu/Square group + Rsqrt group) ---
        warm = const_pool.tile([1, 2], F32)
        nc.vector.memset(warm[:], 1.0)
        warm2 = const_pool.tile([1, 2], F32)
        nc.scalar.activation(warm2[:, 0:1], warm[:, 0:1], ACTF.Relu)
        nc.scalar.activation(warm2[:, 1:2], warm[:, 1:2], ACTF.Sqrt)

        # --- collective warm-up: absorb cold-start + cross-core skew ---
        ccw_in = dram.tile([1, 8], F32)
        ccw_out = dram.tile([N_CORES, 8], F32)
        warm3 = const_pool.tile([1, 8], F32)
        nc.vector.memset(warm3[:], 0.0)
        nc.sync.dma_start(ccw_in[:], warm3[:])
        nc.gpsimd.collective_compute(
            "AllGather", OP.bypass, replica_groups=[list(range(N_CORES))],
            ins=[ccw_in.opt()], outs=[ccw_out.opt()])

        # --- constants (gpsimd DMA queue; xb/S stream on sync queue) ---
        w1s_s = const_pool.tile([2 * C_IN, 128], BF16)
        nc.gpsimd.dma_start(w1s_s[:], w1s[:])
        w2s_s = const_pool.tile([128, 128], BF16)
        nc.gpsimd.dma_start(w2s_s[:], w2s[:])
        identf_s = const_pool.tile([128, 128], F32)
        nc.gpsimd.dma_start(identf_s[:], identf[:])
        w1a_s = const_pool.tile([128, M1], BF16)
        nc.gpsimd.dma_start(w1a_s[:], w1a[:])
        w1b_s = const_pool.tile([128, M1], BF16)
        nc.gpsimd.dma_start(w1b_s[:], w1b[:])
        w1x_s = const_pool.tile([S_SCALAR, M1], BF16)
        nc.gpsimd.dma_start(w1x_s[:], w1x[:])
        mw2t_s = const_pool.tile([M1, M2], BF16)
        nc.gpsimd.dma_start(mw2t_s[:], mw2t[:])
        mw3t_s = const_pool.tile([M2, 64], BF16)
        nc.gpsimd.dma_start(mw3t_s[:], mw3t[:])
        g1_s = const_pool.tile([M1, 1], F32); nc.gpsimd.dma_start(g1_s[:], bn1_g[:])
        bb1_s = const_pool.tile([M1, 1], F32); nc.gpsimd.dma_start(bb1_s[:], bn1_b[:])
        g2_s = const_pool.tile([M2, 1], F32); nc.gpsimd.dma_start(g2_s[:], bn2_g[:])
        bb2_s = const_pool.tile([M2, 1], F32); nc.gpsimd.dma_start(bb2_s[:], bn2_b[:])

        xsT_s = z_pool.tile([S_SCALAR, sl], BF16)
        nc.gpsimd.dma_start(xsT_s[:], xsT[:])

        # --- PE warm-up burst: ~10us of back-to-back matmuls under the
        # initial DMA wait so the loop starts at full HAM clock ---
        if _FLAG("WARM_BURST"):
            wbm = const_pool.tile([128, 512], BF16)
            nc.vector.memset(wbm[:], 0.0)
            pwm = ps_a.tile([128, 512], F32, tag="p1")
            for _ in range(32):
                nc.tensor.matmul(pwm[:, :], w2s_s[:], wbm[:], start=True,
                                 stop=True)

        zq_s = z_pool.tile([128, sl], BF16, tag="zq")  # pooled (junk halves ok)
        v1_s = mlp_pool.tile([M1, sl], BF16, tag="v1")
        scr = mlp_pool.tile([M1, 512], BF16, tag="scr")
        s1p = stat_pool.tile([M1, 2 * nzb], F32, tag="s1p")  # per-block stats

        # early stats gather: blocks 0..nb1-1 AllGathered while the loop is
        # still running; only the last block's stats wait for loop end
        cc1a_in = dram.tile([2 * nb1, M1], F32)
        cc1a_out = dram.tile([2 * nb1 * N_CORES, M1], F32)
        cc1b_in = dram.tile([2, M1], F32)
        cc1b_out = dram.tile([2 * N_CORES, M1], F32)

        def v1_block(b, zpa):
            a, e = 512 * b, min(sl, 512 * (b + 1))
            w = e - a
            # contiguous evict of the pooled bank (junk killed by W1A/W1B)
            if b % 2 == 0:
                nc.vector.tensor_copy(zq_s[:, a:a + w], zpa[:, :w])
            else:
                nc.scalar.activation(zq_s[:, a:a + w], zpa[:, :w], ACTF.Copy)
            hw = w // 2
            pv = ps_a.tile([M1, 512], F32, tag="p1")
            # start=True clears has_written for the whole bank, so only the
            # first matmul of the group may set it
            nc.tensor.matmul(pv[:, 0:hw], w1a_s[:], zq_s[:, a:a + hw],
                             start=True, stop=False)
            nc.tensor.matmul(pv[:, hw:w], w1b_s[:], zq_s[:, a + hw:a + w],
                             start=False, stop=False)
            nc.tensor.matmul(pv[:, 0:w], w1x_s[:], xsT_s[:, a:e],
                             start=False, stop=True)
            if b % 2 == 0:
                nc.vector.tensor_copy(v1_s[:, a:e], pv[:, :w])
            else:
                nc.scalar.activation(v1_s[:, a:e], pv[:, :w], ACTF.Copy)
            nc.vector.tensor_reduce(s1p[:, 2 * b:2 * b + 1], v1_s[:, a:e],
                                    axis=AX.X, op=OP.add)
            nc.scalar.activation(scr[:, :w], v1_s[:, a:e], ACTF.Square,
                                 accum_out=s1p[:, 2 * b + 1:2 * b + 2])
            if b < nb1 - 1 and _FLAG("DUMMY_AG"):
                # dummy collective keeps the cc ring warm mid-loop
                nc.gpsimd.collective_compute(
                    "AllGather", OP.bypass,
                    replica_groups=[list(range(N_CORES))],
                    ins=[ccw_in.opt()], outs=[ccw_out.opt()])
            elif b == nb1 - 1:
                # launch the early stats AllGather under the last block
                tap = ps_a.tile([M1, 512], F32, tag="p1")
                nc.tensor.transpose(tap[0:2 * nb1, 0:M1], s1p[:, 0:2 * nb1],
                                    identf_s[:])
                tas = stat_pool.tile([2 * nb1, M1], F32, tag="tas")
                nc.vector.tensor_copy(tas[:], tap[0:2 * nb1, 0:M1])
                nc.sync.dma_start(cc1a_in[:], tas[:])
                nc.gpsimd.collective_compute(
                    "AllGather", OP.bypass,
                    replica_groups=[list(range(N_CORES))],
                    ins=[cc1a_in.opt()], outs=[cc1a_out.opt()])

        # ---------- main loop ----------
        zpa = None
        for j in range(nchunk):
            if j == 0:
                # small first batch so the PE starts quickly
                xb_t = xb_pool.tile([2 * C_IN, XCHUNK * G], BF16, tag="xb")
                nc.sync.dma_start(xb_t[:, :2 * G], xb[:, 0:2 * G])
                nc.sync.dma_start(xb_t[:, 2 * G:XCHUNK * G],
                                  xb[:, 2 * G:XCHUNK * G])
                s_t = s_pool.tile([128, SCHUNK * 64], BF16, tag="st")
                nc.sync.dma_start(s_t[:, :2 * 64], s_in[:, 0:2 * 64])
                nc.sync.dma_start(s_t[:, 2 * 64:SCHUNK * 64],
                                  s_in[:, 2 * 64:SCHUNK * 64])
            else:
                if j % XCHUNK == 0:
                    nx = min(XCHUNK, nchunk - j)
                    xb_t = xb_pool.tile([2 * C_IN, XCHUNK * G], BF16, tag="xb")
                    nc.sync.dma_start(xb_t[:, :nx * G], xb[:, j * G:(j + nx) * G])
                if j % SCHUNK == 0:
                    ns = min(SCHUNK, nchunk - j)
                    s_t = s_pool.tile([128, SCHUNK * 64], BF16, tag="st")
                    nc.sync.dma_start(s_t[:, :ns * 64],
                                      s_in[:, j * 64:(j + ns) * 64])
            jx = (j % XCHUNK) * G
            js = (j % SCHUNK) * 64
            b = j // ZBLK
            jj = j % ZBLK
            cib = min(ZBLK, nchunk - b * ZBLK)
            if jj == 0:
                zpa = ps_z.tile([128, 512], F32, tag="zpa")

            p1 = ps_a.tile([128, 512], F32, tag="p1")
            nc.tensor.matmul(p1[:, :], w1s_s[:], xb_t[:, jx:jx + G],
                             start=True, stop=True)
            h1_t = h1_pool.tile([128, 512], BF16, tag="h1")
            if j % 2 == 0:
                nc.vector.tensor_scalar(h1_t[:], p1[:], 0.0, None, OP.max)
            else:
                nc.scalar.activation(h1_t[:], p1[:], ACTF.Relu)

            # phi2: h1 k-tile stationary, block-diag W2 moving -> h2^T
            p2 = ps_b.tile([128, 512], F32, tag="p2")
            for t in range(4):
                nc.tensor.matmul(p2[:, 128 * t:128 * t + 128],
                                 h1_t[:, 128 * t:128 * t + 128], w2s_s[:],
                                 start=True, stop=True)
            h2_t = h2_pool.tile([128, 512], BF16, tag="h2")
            if j % 2 == 1:
                nc.vector.tensor_scalar(h2_t[:], p2[:], 0.0, None, OP.max)
            else:
                nc.scalar.activation(h2_t[:], p2[:], ACTF.Relu)

            # pool: h2^T k-tile stationary, S moving (16 slot cols);
            # A slots land in the left half-bank, B slots in the right
            # (strided out AP); junk halves are killed later by W1A/W1B.
            dst = zpa[:, 0:16 * cib].rearrange(
                "p (h j c) -> p h j c", h=2, c=8)[:, :, jj:jj + 1, :]
            for t in range(4):
                nc.tensor.matmul(
                    dst,
                    h2_t[:, 128 * t:128 * t + 128],
                    s_t[:, js + 16 * t: js + 16 * t + 16],
                    start=(t == 0), stop=(t == 3))

            if jj == cib - 1:
                v1_block(b, zpa)

        # ---------- MLP tail ----------
        # last block stats -> [2, M1] -> AllGather (ring is warm)
        t1p = ps_a.tile([M1, 512], F32, tag="p1")
        nc.tensor.transpose(t1p[0:2, 0:M1], s1p[:, 2 * nb1:2 * nzb],
                            identf_s[:])
        t1s = stat_pool.tile([2, M1], F32, tag="t1s")
        nc.vector.tensor_copy(t1s[:], t1p[0:2, 0:M1])
        nc.sync.dma_start(cc1b_in[:], t1s[:])
        nc.gpsimd.collective_compute(
            "AllGather", OP.bypass, replica_groups=[list(range(N_CORES))],
            ins=[cc1b_in.opt()], outs=[cc1b_out.opt()])

        # fold the early gather (blocks 0..nb1-1, all ranks)
        aga = stat_pool.tile([2 * nb1 * N_CORES, M1], F32, tag="aga")
        nc.sync.dma_start(aga[:], cc1a_out[:])
        agap = ps_a.tile([M1, 512], F32, tag="p1")
        na = 2 * nb1 * N_CORES
        nc.tensor.transpose(agap[0:M1, 0:na], aga[:],
                            identf_s[0:na, 0:na])
        agTa = stat_pool.tile([M1, na], F32, tag="agTa")
        nc.vector.tensor_copy(agTa[:], agap[0:M1, 0:na])
        # cols: rank-major, then block, then (sum, sumsq)
        reda = stat_pool.tile([M1, 2 * N_CORES], F32, tag="reda")
        nc.vector.tensor_reduce(
            reda[:].rearrange("p (k r) -> p k r", k=2),
            agTa[:].rearrange("p (r b k) -> p k r b", k=2, b=nb1),
            axis=AX.X, op=OP.add)
        s1ga = stat_pool.tile([M1, 2], F32, tag="s1ga")
        nc.vector.tensor_reduce(
            s1ga[:].rearrange("p (k o) -> p k o", k=2),
            reda[:].rearrange("p (k r) -> p k r", k=2),
            axis=AX.X, op=OP.add)

        # fold the last-block gather and combine
        agb = stat_pool.tile([2 * N_CORES, M1], F32, tag="agb")
        nc.sync.dma_start(agb[:], cc1b_out[:])
        agp1 = ps_a.tile([M1, 512], F32, tag="p1")
        nc.tensor.transpose(agp1[0:M1, 0:16], agb[:],
                            identf_s[0:2 * N_CORES, 0:2 * N_CORES])
        agT1 = stat_pool.tile([M1, 16], F32, tag="agT1")
        nc.vector.tensor_copy(agT1[:], agp1[0:M1, 0:16])
        s1gb = stat_pool.tile([M1, 2], F32, tag="s1gb")
        nc.vector.tensor_reduce(
            s1gb[:].rearrange("p (k o) -> p k o", k=2),
            agT1[:].rearrange("p (r k) -> p k r", k=2),
            axis=AX.X, op=OP.add)
        s1g = stat_pool.tile([M1, 2], F32, tag="s1g")
        nc.vector.tensor_tensor(s1g[:], s1ga[:], s1gb[:], OP.add)

        # BN1 coefficients: sc1 = g/sqrt(var+eps), of1 = b - mean*sc1
        ms1 = stat_pool.tile([M1, 2], F32, tag="ms1")
        nc.scalar.mul(ms1[:], s1g[:], inv_b)
        t1 = stat_pool.tile([M1, 6], F32, tag="t1")
        msq1, var1, iv1, sd1, sc1, of1 = (t1[:, i:i + 1] for i in range(6))
        nc.vector.tensor_tensor(msq1, ms1[:, 0:1], ms1[:, 0:1], OP.mult)
        nc.vector.tensor_scalar(msq1, msq1, -1.0, EPS, OP.mult, OP.add)
        nc.vector.tensor_tensor(var1, ms1[:, 1:2], msq1, OP.add)
        nc.vector.reciprocal(iv1, var1)
        nc.scalar.activation(sd1, iv1, ACTF.Sqrt)
        nc.vector.tensor_tensor(sc1, g1_s[:], sd1, OP.mult)
        nc.vector.tensor_tensor(of1, ms1[:, 0:1], sc1, OP.mult)
        nc.vector.tensor_tensor(of1, bb1_s[:], of1, OP.subtract)

        # empty-slot column of a1 (v1 = 0 there): a1_e = relu(of1)
        a1e = stat_pool.tile([M1, 1], BF16, tag="a1e")
        nc.scalar.activation(a1e[:], of1, ACTF.Relu)
        pve = ps_b.tile([M2, 512], F32, tag="p2")
        nc.tensor.matmul(pve[:, 0:1], mw2t_s[:], a1e[:], start=True, stop=True)
        ve = stat_pool.tile([M2, 2], F32, tag="ve")
        nc.vector.tensor_copy(ve[:, 0:1], pve[:, 0:1])
        nc.vector.tensor_tensor(ve[:, 1:2], ve[:, 0:1], ve[:, 0:1], OP.mult)

        # a1 = relu(sc1*v1 + of1) per block, pipelined into v2 + stats
        a1_s = mlp_pool.tile([M1, sl], BF16, tag="a1")
        v2_s = mlp_pool.tile([M2, sl], BF16, tag="v2")
        s2p = stat_pool.tile([M2, 2 * nzb], F32, tag="s2p")
        for b in range(nzb):
            a, e = 512 * b, min(sl, 512 * (b + 1))
            w = e - a
            nc.scalar.activation(a1_s[:, a:e], v1_s[:, a:e], ACTF.Relu,
                                 bias=of1, scale=sc1)
            pv = ps_b.tile([M2, 512], F32, tag="p2")
            nc.tensor.matmul(pv[:, :w], mw2t_s[:], a1_s[:, a:e],
                             start=True, stop=True)
            nc.vector.tensor_copy(v2_s[:, a:e], pv[:, :w])
            nc.vector.tensor_reduce(s2p[:, 2 * b:2 * b + 1], v2_s[:, a:e],
                                    axis=AX.X, op=OP.add)
            nc.scalar.activation(scr[0:M2, :w], v2_s[:, a:e], ACTF.Square,
                                 accum_out=s2p[:, 2 * b + 1:2 * b + 2])

        s2 = stat_pool.tile([M2, 2], F32, tag="s2")
        s2r = s2p[:].rearrange("p (b k) -> p k b", k=2)
        nc.vector.tensor_reduce(s2[:, 0:1], s2r[:, 0:1, :], axis=AX.X,
                                op=OP.add)
        nc.vector.tensor_reduce(s2[:, 1:2], s2r[:, 1:2, :], axis=AX.X,
                                op=OP.add)

        t2p = ps_b.tile([M2, 512], F32, tag="p2")
        nc.tensor.transpose(t2p[0:2, 0:M2], s2[:], identf_s[0:M2, 0:M2])
        t2s = stat_pool.tile([2, M2], F32, tag="t2s")
        nc.vector.tensor_copy(t2s[:], t2p[0:2, 0:M2])
        cc2_in = dram.tile([2, M2], F32)
        cc2_out = dram.tile([2 * N_CORES, M2], F32)
        nc.sync.dma_start(cc2_in[:], t2s[:])
        nc.gpsimd.collective_compute(
            "AllGather", OP.bypass, replica_groups=[list(range(N_CORES))],
            ins=[cc2_in.opt()], outs=[cc2_out.opt()])
        ag2 = stat_pool.tile([2 * N_CORES, M2], F32, tag="ag2")
        nc.sync.dma_start(ag2[:], cc2_out[:])
        agp2 = ps_b.tile([M2, 512], F32, tag="p2")
        nc.tensor.transpose(agp2[0:M2, 0:16], ag2[:],
                            identf_s[0:2 * N_CORES, 0:2 * N_CORES])
        agT2 = stat_pool.tile([M2, 16], F32, tag="agT2")
        nc.vector.tensor_copy(agT2[:], agp2[0:M2, 0:16])
        s2g = stat_pool.tile([M2, 2], F32, tag="s2g")
        agr2 = agT2[:].rearrange("p (r k) -> p k r", k=2)
        nc.vector.tensor_reduce(s2g[:, 0:1], agr2[:, 0:1, :], axis=AX.X,
                                op=OP.add)
        nc.vector.tensor_reduce(s2g[:, 1:2], agr2[:, 1:2, :], axis=AX.X,
                                op=OP.add)

        # empty-slot correction then BN2 coefficients
        s2c = stat_pool.tile([M2, 2], F32, tag="s2c")
        nc.vector.scalar_tensor_tensor(
            s2c[:, 0:1], ve[:, 0:1], -n_empty, s2g[:, 0:1], OP.mult, OP.add)
        nc.vector.scalar_tensor_tensor(
            s2c[:, 1:2], ve[:, 1:2], -n_empty, s2g[:, 1:2], OP.mult, OP.add)
        ms2 = stat_pool.tile([M2, 2], F32, tag="ms2")
        nc.scalar.mul(ms2[:], s2c[:], inv_b)
        t2 = stat_pool.tile([M2, 6], F32, tag="t2")
        msq2, var2, iv2, sd2, sc2, of2 = (t2[:, i:i + 1] for i in range(6))
        nc.vector.tensor_tensor(msq2, ms2[:, 0:1], ms2[:, 0:1], OP.mult)
        nc.vector.tensor_scalar(msq2, msq2, -1.0, EPS, OP.mult, OP.add)
        nc.vector.tensor_tensor(var2, ms2[:, 1:2], msq2, OP.add)
        nc.vector.reciprocal(iv2, var2)
        nc.scalar.activation(sd2, iv2, ACTF.Sqrt)
        nc.vector.tensor_tensor(sc2, g2_s[:], sd2, OP.mult)
        nc.vector.tensor_tensor(of2, ms2[:, 0:1], sc2, OP.mult)
        nc.vector.tensor_tensor(of2, bb2_s[:], of2, OP.subtract)

        # a2 + final matmul per block; block b lands on y partition b so the
        # final DMA is one multi-partition transfer
        a2_s = mlp_pool.tile([M2, sl], BF16, tag="a2")
        y_s = mlp_pool.tile([8, 512], F32, tag="y")
        pv8 = ps_a.tile([8, 512], F32, tag="p1")
        for b in range(nzb):
            a, e = 512 * b, min(sl, 512 * (b + 1))
            w = e - a
            nc.scalar.activation(a2_s[:, a:e], v2_s[:, a:e], ACTF.Relu,
                                 bias=of2, scale=sc2)
            # stationary slice b has w3 only in column b: block b accumulates
            # onto y row b (other rows += 0)
            nc.tensor.matmul(pv8[:, :w], mw3t_s[:, 8 * b:8 * b + 8],
                             a2_s[:, a:e], start=(b == 0),
                             stop=(b == nzb - 1))
        nc.vector.tensor_copy(y_s[:], pv8[:])
        nc.sync.dma_start(y_out[0:nzb, :], y_s[0:nzb, :])

    nc.compile()
    return nc


def _mw3_blocks(mlp_w3):
    m = np.zeros((M2, 64), dtype=np.float32)
    for b in range(8):
        m[:, 8 * b + b] = mlp_w3[0]
    return np.ascontiguousarray(m.astype(ml_dtypes.bfloat16))


_CACHE = {}


def kernel(**inputs) -> np.ndarray:
    x = np.asarray(inputs["x_set"], np.float32)[0]        # [16, T]
    x_scalar = np.asarray(inputs["x_scalar"], np.float32)  # [B, 8]
    lengths = np.asarray(inputs["sample_indices"])[0].astype(np.int64)
    b_total = x_scalar.shape[0]
    offsets = np.concatenate([[0], np.cumsum(lengths)[:-1]])

    groups, ng = _plan_groups(lengths, b_total)
    nchunk = ng // 2
    sl = nchunk * 16
    per_core = [
        _prep_core(x, x_scalar, lengths, offsets, groups[c], ng)
        for c in range(N_CORES)
    ]

    b1 = np.asarray(inputs["phi_b1"], np.float32)
    b2 = np.asarray(inputs["phi_b2"], np.float32)
    assert np.all(b1 == 0.0) and np.all(b2 == 0.0), \
        "nonzero phi bias path not implemented"
    w1s = np.zeros((2 * C_IN, 128), dtype=np.float32)
    w1s[0:C_IN, 0:F] = np.asarray(inputs["phi_w1"], np.float32).T
    w1s[C_IN:2 * C_IN, F:128] = np.asarray(inputs["phi_w1"], np.float32).T
    w2s = np.zeros((128, 128), dtype=np.float32)
    w2s[0:F, 0:F] = np.asarray(inputs["phi_w2"], np.float32).T
    w2s[F:128, F:128] = np.asarray(inputs["phi_w2"], np.float32).T
    mw1 = np.asarray(inputs["mlp_w1"], np.float32)        # [128, 72]
    w1a = np.zeros((128, M1), dtype=np.float32)
    w1a[0:F, :] = mw1[:, 0:F].T
    w1b = np.zeros((128, M1), dtype=np.float32)
    w1b[F:128, :] = mw1[:, 0:F].T
    w1x = np.ascontiguousarray(mw1[:, F:F + S_SCALAR].T)
    consts = {
        "w1s": np.ascontiguousarray(w1s.astype(ml_dtypes.bfloat16)),
        "w2s": np.ascontiguousarray(w2s.astype(ml_dtypes.bfloat16)),
        "identf": np.ascontiguousarray(np.eye(128, dtype=np.float32)),
        "w1a": np.ascontiguousarray(w1a.astype(ml_dtypes.bfloat16)),
        "w1b": np.ascontiguousarray(w1b.astype(ml_dtypes.bfloat16)),
        "w1x": np.ascontiguousarray(w1x.astype(ml_dtypes.bfloat16)),
        "mw2t": np.ascontiguousarray(
            np.asarray(inputs["mlp_w2"], np.float32).T.astype(ml_dtypes.bfloat16)),
        "mw3t": _mw3_blocks(np.asarray(inputs["mlp_w3"], np.float32)),
        "bn1_g": np.asarray(inputs["bn1_g"], np.float32).reshape(M1, 1),
        "bn1_b": np.asarray(inputs["bn1_b"], np.float32).reshape(M1, 1),
        "bn2_g": np.asarray(inputs["bn2_g"], np.float32).reshape(M2, 1),
        "bn2_b": np.asarray(inputs["bn2_b"], np.float32).reshape(M2, 1),
    }

    key = (ng, sl, b_total)
    if key not in _CACHE:
        _CACHE[key] = _build_nc(ng, sl, b_total)
    nc = _CACHE[key]

    in_maps = []
    for pc in per_core:
        m = {"xb": pc["xb"], "S": pc["S"], "xsT": pc["xsT"]}
        m.update(consts)
        in_maps.append(m)

    res = bass_utils.run_bass_kernel_spmd(
        nc, in_maps, core_ids=list(range(N_CORES)))

    b3 = float(np.asarray(inputs["mlp_b3"], np.float32).reshape(-1)[0])
    y = np.zeros((b_total, 1), dtype=np.float32)
    for c, pc in enumerate(per_core):
        u = res.results[c]["y"].reshape(-1)[:sl].astype(np.float64) + b3
        ys = 1.0 / (1.0 + np.exp(-u))
        se = pc["slot_events"]
        mask = se >= 0
        y[se[mask], 0] = ys[mask].astype(np.float32)
    return y


# revision 30
# speedup vs baseline: 1.0921x; 1.0921x over previous
"""DeepSets ensemble (segment mean-pool + BN MLP) on 8 TRN2 NeuronCores.

Strategy (data-parallel, per sharding hint):
 - events split 1024/core; each core's points bin-packed (FFD) into 512-pt
   groups of whole events (zero-padded); groups paired A/B so phi runs on
   [32,512] bf16 tiles with 128 output partitions (2x64 feats).
 - per j (1024 pts): phi1 matmul (w1s stationary), relu-evict h1 (DVE/ACT
   alternating), phi2 as 4 k-tile matmuls with h1 stationary producing
   h2^T (point-pair major), relu-evict h2, pooling as one 16-col matmul
   per k-tile (h2^T stationary, S moving) accumulating into a PSUM bank
   that holds ZBLK j's, with A slots in the left half-bank and B slots in
   the right half (strided matmul output AP).
 - S carries 1/len instead of 0/1 so pooling directly yields the mean;
   the bank evicts with one contiguous copy and v1 = W1 @ z runs as three
   accumulated matmuls (W1A/W1B with zeroed halves kill the junk
   quadrants, plus the x_scalar term); per-block BN statistics run inside
   the loop so only an AllGather remains at the end.
 - BatchNorm stats via PE-transposed [2,128] tensors + AllGather + on-core
   fold; a tiny warm-up AllGather at kernel start absorbs the collective
   cold-start/skew; empty-slot contributions corrected analytically;
   MLP in bf16; sigmoid + final bias applied on host.
"""
import sys
import numpy as np
import ml_dtypes
from contextlib import ExitStack

sys.path.insert(0, "/opt/trn_rl_repo")

import os
import concourse.bacc as bacc


def _FLAG(name):
    return os.environ.get(name, "1") == "1"

import concourse.tile as tile
from concourse import mybir
from concourse import bass_utils

BF16 = mybir.dt.bfloat16
F32 = mybir.dt.float32
AX = mybir.AxisListType
OP = mybir.AluOpType
ACTF = mybir.ActivationFunctionType

N_CORES = 8
C_IN = 16
F = 64
S_SCALAR = 8
M1, M2 = 128, 64
G = 512
SLOTS = 8
EPS = 1e-5

XCHUNK = 8
SCHUNK = 8
ZBLK = 32          # j per pooled-psum bank (32*16 = 512 cols)


def _plan_groups(lengths, b_total):
    e_per_core = b_total // N_CORES
    cores = []
    for c in range(N_CORES):
        evs = sorted(range(c * e_per_core, (c + 1) * e_per_core),
                     key=lambda e: -int(lengths[e]))
        groups, space = [], []
        for e in evs:
            l = int(lengths[e])
            assert 0 < l <= G
            placed = False
            for gi in range(len(groups)):
                if space[gi] >= l and len(groups[gi]) < SLOTS:
                    groups[gi].append(e)
                    space[gi] -= l
                    placed = True
                    break
            if not placed:
                groups.append([e])
                space.append(G - l)
        cores.append(groups)
    ng = max(len(g) for g in cores)
    if ng % 8:
        ng += 8 - ng % 8
    for g in cores:
        while len(g) < ng:
            g.append([])
    return cores, ng


def _slot_col(j, s, nchunk):
    """Column of slot (chunk j, slot s<16) in the z layout: per ZBLK block,
    A slots (s<8) fill the left half, B slots the right half."""
    b, jj = divmod(j, ZBLK)
    cib = min(ZBLK, nchunk - b * ZBLK)          # chunks in this block
    half = 0 if s < 8 else 1
    return b * ZBLK * 16 + half * 8 * cib + jj * 8 + (s % 8)


def _prep_core(x, x_scalar, lengths, offsets, groups, ng):
    nchunk = ng // 2
    sl = nchunk * 16
    xb = np.zeros((2 * C_IN, nchunk * G), dtype=np.float32)
    s_mat = np.zeros((128, nchunk * 64), dtype=np.float32)
    xsT = np.zeros((S_SCALAR, sl), dtype=np.float32)
    slot_events = np.full(sl, -1, dtype=np.int64)
    for j in range(nchunk):
        for half, g_idx in ((0, j), (1, nchunk + j)):
            evs = groups[g_idx]
            col0 = G * j
            row0 = C_IN * half
            pt = 0
            for i, e in enumerate(evs):
                l = int(lengths[e])
                o = int(offsets[e])
                xb[row0:row0 + C_IN, col0 + pt: col0 + pt + l] = x[:, o:o + l]
                # S: point-pair rows within k-tile; slot cols (A:0-7, B:8-15)
                # value 1/len so pooling directly produces the mean
                p_arr = np.arange(pt, pt + l)
                s = i + 8 * half
                s_mat[p_arr % 128, j * 64 + (p_arr // 128) * 16 + s] = 1.0 / l
                slot = _slot_col(j, s, nchunk)
                xsT[:, slot] = x_scalar[e]
                slot_events[slot] = e
                pt += l
    return {
        "xb": np.ascontiguousarray(xb.astype(ml_dtypes.bfloat16)),
        "S": np.ascontiguousarray(s_mat.astype(ml_dtypes.bfloat16)),
        "xsT": np.ascontiguousarray(xsT.astype(ml_dtypes.bfloat16)),
        "slot_events": slot_events,
    }


def _build_nc(ng, sl, b_total):
    nchunk = ng // 2
    n_empty = float(N_CORES * sl - b_total)
    inv_b = 1.0 / float(b_total)

    nc = bacc.Bacc("TRN2", target_bir_lowering=False, debug=False,
                   num_devices=N_CORES)

    def din(name, shape, dt):
        return nc.dram_tensor(name, shape, dt, kind="ExternalInput").ap()

    xb = din("xb", [2 * C_IN, nchunk * G], BF16)
    s_in = din("S", [128, nchunk * 64], BF16)
    xsT = din("xsT", [S_SCALAR, sl], BF16)
    w1s = din("w1s", [2 * C_IN, 128], BF16)
    w2s = din("w2s", [128, 128], BF16)
    identf = din("identf", [128, 128], F32)
    w1a = din("w1a", [128, M1], BF16)       # rows 0:64 = mw1[:, :64].T
    w1b = din("w1b", [128, M1], BF16)       # rows 64:128 = mw1[:, :64].T
    w1x = din("w1x", [S_SCALAR, M1], BF16)  # mw1[:, 64:72].T
    mw2t = din("mw2t", [M1, M2], BF16)
    mw3t = din("mw3t", [M2, 64], BF16)  # per-block slices: slice b holds w3
                                        # in col b so block b lands on y row b
    bn1_g = din("bn1_g", [M1, 1], F32)
    bn1_b = din("bn1_b", [M1, 1], F32)
    bn2_g = din("bn2_g", [M2, 1], F32)
    bn2_b = din("bn2_b", [M2, 1], F32)

    nzb = (nchunk + ZBLK - 1) // ZBLK   # number of z blocks (512 cols each)
    assert 3 <= nzb <= 8
    # early gather covers blocks 0..nzb-3 (done by ~75% of the loop, so the
    # collective hides under it); the last TWO blocks gather after the loop
    nb1 = nzb - 2

    y_out = nc.dram_tensor("y", [8, 512], F32, kind="ExternalOutput").ap()

    with tile.TileContext(nc) as tc, ExitStack() as ctx:
        const_pool = ctx.enter_context(tc.tile_pool(name="const", bufs=1))
        xb_pool = ctx.enter_context(tc.tile_pool(name="xb", bufs=2))
        s_pool = ctx.enter_context(tc.tile_pool(name="spool", bufs=2))
        h1_pool = ctx.enter_context(tc.tile_pool(name="h1", bufs=4))
        h2_pool = ctx.enter_context(tc.tile_pool(name="h2", bufs=4))
        z_pool = ctx.enter_context(tc.tile_pool(name="z", bufs=1))
        mlp_pool = ctx.enter_context(tc.tile_pool(name="mlp", bufs=1))
        stat_pool = ctx.enter_context(tc.tile_pool(name="stat", bufs=1))
        ps_a = ctx.enter_context(tc.tile_pool(name="psa", bufs=3, space="PSUM"))
        ps_b = ctx.enter_context(tc.tile_pool(name="psb", bufs=3, space="PSUM"))
        ps_z = ctx.enter_context(tc.tile_pool(name="psz", bufs=2, space="PSUM"))
        dram = ctx.enter_context(tc.tile_pool(name="dram", bufs=1, space="DRAM"))

        # --- activation-table preloads (Relu/Square group + Rsqrt group) ---
        warm = const_pool.tile([1, 2], F32)
        nc.vector.memset(warm[:], 1.0)
        warm2 = const_pool.tile([1, 2], F32)
        nc.scalar.activation(warm2[:, 0:1], warm[:, 0:1], ACTF.Relu)
        nc.scalar.activation(warm2[:, 1:2], warm[:, 1:2], ACTF.Sqrt)

        # --- collective warm-up: absorb cold-start + cross-core skew ---
        ccw_in = dram.tile([1, 8], F32)
        ccw_out = dram.tile([N_CORES, 8], F32)
        warm3 = const_pool.tile([1, 8], F32)
        nc.vector.memset(warm3[:], 0.0)
        nc.sync.dma_start(ccw_in[:], warm3[:])
        nc.gpsimd.collective_compute(
            "AllGather", OP.bypass, replica_groups=[list(range(N_CORES))],
            ins=[ccw_in.opt()], outs=[ccw_out.opt()])

        # --- constants (gpsimd DMA queue; xb/S stream on sync queue) ---
        w1s_s = const_pool.tile([2 * C_IN, 128], BF16)
        nc.gpsimd.dma_start(w1s_s[:], w1s[:])
        w2s_s = const_pool.tile([128, 128], BF16)
        nc.gpsimd.dma_start(w2s_s[:], w2s[:])
        identf_s = const_pool.tile([128, 128], F32)
        nc.gpsimd.dma_start(identf_s[:], identf[:])
        w1a_s = const_pool.tile([128, M1], BF16)
        nc.gpsimd.dma_start(w1a_s[:], w1a[:])
        w1b_s = const_pool.tile([128, M1], BF16)
        nc.gpsimd.dma_start(w1b_s[:], w1b[:])
        w1x_s = const_pool.tile([S_SCALAR, M1], BF16)
        nc.gpsimd.dma_start(w1x_s[:], w1x[:])
        mw2t_s = const_pool.tile([M1, M2], BF16)
        nc.gpsimd.dma_start(mw2t_s[:], mw2t[:])
        mw3t_s = const_pool.tile([M2, 64], BF16)
        nc.gpsimd.dma_start(mw3t_s[:], mw3t[:])
        g1_s = const_pool.tile([M1, 1], F32); nc.gpsimd.dma_start(g1_s[:], bn1_g[:])
        bb1_s = const_pool.tile([M1, 1], F32); nc.gpsimd.dma_start(bb1_s[:], bn1_b[:])
        g2_s = const_pool.tile([M2, 1], F32); nc.gpsimd.dma_start(g2_s[:], bn2_g[:])
        bb2_s = const_pool.tile([M2, 1], F32); nc.gpsimd.dma_start(bb2_s[:], bn2_b[:])

        xsT_s = z_pool.tile([S_SCALAR, sl], BF16)
        nc.gpsimd.dma_start(xsT_s[:], xsT[:])



        zq_s = z_pool.tile([128, sl], BF16, tag="zq")  # pooled (junk halves ok)
        v1_s = mlp_pool.tile([M1, sl], BF16, tag="v1")
        scr = mlp_pool.tile([M1, 512], BF16, tag="scr")
        s1p = stat_pool.tile([M1, 2 * nzb], F32, tag="s1p")  # per-block stats

        # early stats gather: blocks 0..nb1-1 AllGathered while the loop is
        # still running; only the last block's stats wait for loop end
        cc1a_in = dram.tile([2 * nb1, M1], F32)
        cc1a_out = dram.tile([2 * nb1 * N_CORES, M1], F32)
        cc1b_in = dram.tile([4, M1], F32)
        cc1b_out = dram.tile([4 * N_CORES, M1], F32)

        def v1_block(b, zpa):
            a, e = 512 * b, min(sl, 512 * (b + 1))
            w = e - a
            # contiguous evict of the pooled bank (junk killed by W1A/W1B)
            if b % 2 == 0:
                nc.vector.tensor_copy(zq_s[:, a:a + w], zpa[:, :w])
            else:
                nc.scalar.activation(zq_s[:, a:a + w], zpa[:, :w], ACTF.Copy)
            hw = w // 2
            pv = ps_a.tile([M1, 512], F32, tag="p1")
            # start=True clears has_written for the whole bank, so only the
            # first matmul of the group may set it
            nc.tensor.matmul(pv[:, 0:hw], w1a_s[:], zq_s[:, a:a + hw],
                             start=True, stop=False)
            nc.tensor.matmul(pv[:, hw:w], w1b_s[:], zq_s[:, a + hw:a + w],
                             start=False, stop=False)
            nc.tensor.matmul(pv[:, 0:w], w1x_s[:], xsT_s[:, a:e],
                             start=False, stop=True)
            if b % 2 == 0:
                nc.vector.tensor_copy(v1_s[:, a:e], pv[:, :w])
            else:
                nc.scalar.activation(v1_s[:, a:e], pv[:, :w], ACTF.Copy)
            nc.vector.tensor_reduce(s1p[:, 2 * b:2 * b + 1], v1_s[:, a:e],
                                    axis=AX.X, op=OP.add)
            nc.scalar.activation(scr[:, :w], v1_s[:, a:e], ACTF.Square,
                                 accum_out=s1p[:, 2 * b + 1:2 * b + 2])
            if (b < nb1 - 1 or b == nzb - 2) and _FLAG("DUMMY_AG"):
                # dummy collective: keeps the cc ring warm mid-loop, and the
                # one at block nzb-2 aligns the cores shortly before loop end
                # so the post-loop gather doesn't eat the full core skew
                nc.gpsimd.collective_compute(
                    "AllGather", OP.bypass,
                    replica_groups=[list(range(N_CORES))],
                    ins=[ccw_in.opt()], outs=[ccw_out.opt()])
            if b == nb1 - 1:
                # launch the early stats AllGather under the last block
                tap = ps_a.tile([M1, 512], F32, tag="p1")
                nc.tensor.transpose(tap[0:2 * nb1, 0:M1], s1p[:, 0:2 * nb1],
                                    identf_s[:])
                tas = stat_pool.tile([2 * nb1, M1], F32, tag="tas")
                nc.vector.tensor_copy(tas[:], tap[0:2 * nb1, 0:M1])
                nc.sync.dma_start(cc1a_in[:], tas[:])
                nc.gpsimd.collective_compute(
                    "AllGather", OP.bypass,
                    replica_groups=[list(range(N_CORES))],
                    ins=[cc1a_in.opt()], outs=[cc1a_out.opt()])

        # ---------- main loop ----------
        zpa = None
        for j in range(nchunk):
            if j == 0:
                # small first batch so the PE starts quickly
                xb_t = xb_pool.tile([2 * C_IN, XCHUNK * G], BF16, tag="xb")
                nc.sync.dma_start(xb_t[:, :2 * G], xb[:, 0:2 * G])
                nc.sync.dma_start(xb_t[:, 2 * G:XCHUNK * G],
                                  xb[:, 2 * G:XCHUNK * G])
                s_t = s_pool.tile([128, SCHUNK * 64], BF16, tag="st")
                nc.sync.dma_start(s_t[:, :2 * 64], s_in[:, 0:2 * 64])
                nc.sync.dma_start(s_t[:, 2 * 64:SCHUNK * 64],
                                  s_in[:, 2 * 64:SCHUNK * 64])
            else:
                if j % XCHUNK == 0:
                    nx = min(XCHUNK, nchunk - j)
                    xb_t = xb_pool.tile([2 * C_IN, XCHUNK * G], BF16, tag="xb")
                    nc.sync.dma_start(xb_t[:, :nx * G], xb[:, j * G:(j + nx) * G])
                if j % SCHUNK == 0:
                    ns = min(SCHUNK, nchunk - j)
                    s_t = s_pool.tile([128, SCHUNK * 64], BF16, tag="st")
                    nc.sync.dma_start(s_t[:, :ns * 64],
                                      s_in[:, j * 64:(j + ns) * 64])
            jx = (j % XCHUNK) * G
            js = (j % SCHUNK) * 64
            b = j // ZBLK
            jj = j % ZBLK
            cib = min(ZBLK, nchunk - b * ZBLK)
            if jj == 0:
                zpa = ps_z.tile([128, 512], F32, tag="zpa")

            p1 = ps_a.tile([128, 512], F32, tag="p1")
            nc.tensor.matmul(p1[:, :], w1s_s[:], xb_t[:, jx:jx + G],
                             start=True, stop=True)
            h1_t = h1_pool.tile([128, 512], BF16, tag="h1")
            if j % 2 == 0:
                nc.vector.tensor_scalar(h1_t[:], p1[:], 0.0, None, OP.max)
            else:
                nc.scalar.activation(h1_t[:], p1[:], ACTF.Relu)

            # phi2: h1 k-tile stationary, block-diag W2 moving -> h2^T
            p2 = ps_b.tile([128, 512], F32, tag="p2")
            for t in range(4):
                nc.tensor.matmul(p2[:, 128 * t:128 * t + 128],
                                 h1_t[:, 128 * t:128 * t + 128], w2s_s[:],
                                 start=True, stop=True)
            h2_t = h2_pool.tile([128, 512], BF16, tag="h2")
            if j % 2 == 1:
                nc.vector.tensor_scalar(h2_t[:], p2[:], 0.0, None, OP.max)
            else:
                nc.scalar.activation(h2_t[:], p2[:], ACTF.Relu)

            # pool: h2^T k-tile stationary, S moving (16 slot cols);
            # A slots land in the left half-bank, B slots in the right
            # (strided out AP); junk halves are killed later by W1A/W1B.
            dst = zpa[:, 0:16 * cib].rearrange(
                "p (h j c) -> p h j c", h=2, c=8)[:, :, jj:jj + 1, :]
            for t in range(4):
                nc.tensor.matmul(
                    dst,
                    h2_t[:, 128 * t:128 * t + 128],
                    s_t[:, js + 16 * t: js + 16 * t + 16],
                    start=(t == 0), stop=(t == 3))

            if jj == cib - 1:
                v1_block(b, zpa)

        # ---------- MLP tail ----------
        # last two blocks' stats -> [4, M1] -> AllGather (cores were just
        # aligned by the dummy collective at block nzb-2)
        t1p = ps_a.tile([M1, 512], F32, tag="p1")
        nc.tensor.transpose(t1p[0:4, 0:M1], s1p[:, 2 * nb1:2 * nzb],
                            identf_s[:])
        t1s = stat_pool.tile([4, M1], F32, tag="t1s")
        nc.vector.tensor_copy(t1s[:], t1p[0:4, 0:M1])
        nc.sync.dma_start(cc1b_in[:], t1s[:])
        nc.gpsimd.collective_compute(
            "AllGather", OP.bypass, replica_groups=[list(range(N_CORES))],
            ins=[cc1b_in.opt()], outs=[cc1b_out.opt()])

        def fold_stats(cc_out, nb, tag):
            # gathered [2*nb*N_CORES, M1] (rank-major, then block, then
            # (sum, sumsq)) -> [M1, 2]
            na = 2 * nb * N_CORES
            ag = stat_pool.tile([na, M1], F32, tag=tag + "g")
            nc.sync.dma_start(ag[:], cc_out[:])
            agp = ps_a.tile([M1, 512], F32, tag="p1")
            nc.tensor.transpose(agp[0:M1, 0:na], ag[:], identf_s[0:na, 0:na])
            agT = stat_pool.tile([M1, na], F32, tag=tag + "T")
            nc.vector.tensor_copy(agT[:], agp[0:M1, 0:na])
            red = stat_pool.tile([M1, 2 * N_CORES], F32, tag=tag + "r")
            nc.vector.tensor_reduce(
                red[:].rearrange("p (k r) -> p k r", k=2),
                agT[:].rearrange("p (r b k) -> p k r b", k=2, b=nb),
                axis=AX.X, op=OP.add)
            out = stat_pool.tile([M1, 2], F32, tag=tag + "s")
            nc.vector.tensor_reduce(
                out[:].rearrange("p (k o) -> p k o", k=2),
                red[:].rearrange("p (k r) -> p k r", k=2),
                axis=AX.X, op=OP.add)
            return out

        s1ga = fold_stats(cc1a_out, nb1, "fa")
        s1gb = fold_stats(cc1b_out, 2, "fb")
        s1g = stat_pool.tile([M1, 2], F32, tag="s1g")
        nc.vector.tensor_tensor(s1g[:], s1ga[:], s1gb[:], OP.add)

        # BN1 coefficients: sc1 = g/sqrt(var+eps), of1 = b - mean*sc1
        ms1 = stat_pool.tile([M1, 2], F32, tag="ms1")
        nc.scalar.mul(ms1[:], s1g[:], inv_b)
        t1 = stat_pool.tile([M1, 6], F32, tag="t1")
        msq1, var1, iv1, sd1, sc1, of1 = (t1[:, i:i + 1] for i in range(6))
        nc.vector.tensor_tensor(msq1, ms1[:, 0:1], ms1[:, 0:1], OP.mult)
        nc.vector.tensor_scalar(msq1, msq1, -1.0, EPS, OP.mult, OP.add)
        nc.vector.tensor_tensor(var1, ms1[:, 1:2], msq1, OP.add)
        nc.vector.reciprocal(iv1, var1)
        nc.scalar.activation(sd1, iv1, ACTF.Sqrt)
        nc.vector.tensor_tensor(sc1, g1_s[:], sd1, OP.mult)
        nc.vector.tensor_tensor(of1, ms1[:, 0:1], sc1, OP.mult)
        nc.vector.tensor_tensor(of1, bb1_s[:], of1, OP.subtract)

        # empty-slot column of a1 (v1 = 0 there): a1_e = relu(of1)
        a1e = stat_pool.tile([M1, 1], BF16, tag="a1e")
        nc.scalar.activation(a1e[:], of1, ACTF.Relu)
        pve = ps_b.tile([M2, 512], F32, tag="p2")
        nc.tensor.matmul(pve[:, 0:1], mw2t_s[:], a1e[:], start=True, stop=True)
        ve = stat_pool.tile([M2, 2], F32, tag="ve")
        nc.vector.tensor_copy(ve[:, 0:1], pve[:, 0:1])
        nc.vector.tensor_tensor(ve[:, 1:2], ve[:, 0:1], ve[:, 0:1], OP.mult)

        # a1 = relu(sc1*v1 + of1) per block, pipelined into v2 + stats
        a1_s = mlp_pool.tile([M1, sl], BF16, tag="a1")
        v2_s = mlp_pool.tile([M2, sl], BF16, tag="v2")
        s2p = stat_pool.tile([M2, 2 * nzb], F32, tag="s2p")
        for b in range(nzb):
            a, e = 512 * b, min(sl, 512 * (b + 1))
            w = e - a
            nc.scalar.activation(a1_s[:, a:e], v1_s[:, a:e], ACTF.Relu,
                                 bias=of1, scale=sc1)
            pv = ps_b.tile([M2, 512], F32, tag="p2")
            nc.tensor.matmul(pv[:, :w], mw2t_s[:], a1_s[:, a:e],
                             start=True, stop=True)
            nc.vector.tensor_copy(v2_s[:, a:e], pv[:, :w])
            nc.vector.tensor_reduce(s2p[:, 2 * b:2 * b + 1], v2_s[:, a:e],
                                    axis=AX.X, op=OP.add)
            nc.scalar.activation(scr[0:M2, :w], v2_s[:, a:e], ACTF.Square,
                                 accum_out=s2p[:, 2 * b + 1:2 * b + 2])

        s2 = stat_pool.tile([M2, 2], F32, tag="s2")
        s2r = s2p[:].rearrange("p (b k) -> p k b", k=2)
        nc.vector.tensor_reduce(s2[:, 0:1], s2r[:, 0:1, :], axis=AX.X,
                                op=OP.add)
        nc.vector.tensor_reduce(s2[:, 1:2], s2r[:, 1:2, :], axis=AX.X,
                                op=OP.add)

        t2p = ps_b.tile([M2, 512], F32, tag="p2")
        nc.tensor.transpose(t2p[0:2, 0:M2], s2[:], identf_s[0:M2, 0:M2])
        t2s = stat_pool.tile([2, M2], F32, tag="t2s")
        nc.vector.tensor_copy(t2s[:], t2p[0:2, 0:M2])
        cc2_in = dram.tile([2, M2], F32)
        cc2_out = dram.tile([2 * N_CORES, M2], F32)
        nc.sync.dma_start(cc2_in[:], t2s[:])
        nc.gpsimd.collective_compute(
            "AllGather", OP.bypass, replica_groups=[list(range(N_CORES))],
            ins=[cc2_in.opt()], outs=[cc2_out.opt()])
        ag2 = stat_pool.tile([2 * N_CORES, M2], F32, tag="ag2")
        nc.sync.dma_start(ag2[:], cc2_out[:])
        agp2 = ps_b.tile([M2, 512], F32, tag="p2")
        nc.tensor.transpose(agp2[0:M2, 0:16], ag2[:],
                            identf_s[0:2 * N_CORES, 0:2 * N_CORES])
        agT2 = stat_pool.tile([M2, 16], F32, tag="agT2")
        nc.vector.tensor_copy(agT2[:], agp2[0:M2, 0:16])
        s2g = stat_pool.tile([M2, 2], F32, tag="s2g")
        agr2 = agT2[:].rearrange("p (r k) -> p k r", k=2)
        nc.vector.tensor_reduce(s2g[:, 0:1], agr2[:, 0:1, :], axis=AX.X,
                                op=OP.add)
        nc.vector.tensor_reduce(s2g[:, 1:2], agr2[:, 1:2, :], axis=AX.X,
                                op=OP.add)

        # empty-slot correction then BN2 coefficients
        s2c = stat_pool.tile([M2, 2], F32, tag="s2c")
        nc.vector.scalar_tensor_tensor(
            s2c[:, 0:1], ve[:, 0:1], -n_empty, s2g[:, 0:1], OP.mult, OP.add)
        nc.vector.scalar_tensor_tensor(
            s2c[:, 1:2], ve[:, 1:2], -n_empty, s2g[:, 1:2], OP.mult, OP.add)
        ms2 = stat_pool.tile([M2, 2], F32, tag="ms2")
        nc.scalar.mul(ms2[:], s2c[:], inv_b)
        t2 = stat_pool.tile([M2, 6], F32, tag="t2")
        msq2, var2, iv2, sd2, sc2, of2 = (t2[:, i:i + 1] for i in range(6))
        nc.vector.tensor_tensor(msq2, ms2[:, 0:1], ms2[:, 0:1], OP.mult)
        nc.vector.tensor_scalar(msq2, msq2, -1.0, EPS, OP.mult, OP.add)
        nc.vector.tensor_tensor(var2, ms2[:, 1:2], msq2, OP.add)
        nc.vector.reciprocal(iv2, var2)
        nc.scalar.activation(sd2, iv2, ACTF.Sqrt)
        nc.vector.tensor_tensor(sc2, g2_s[:], sd2, OP.mult)
        nc.vector.tensor_tensor(of2, ms2[:, 0:1], sc2, OP.mult)
        nc.vector.tensor_tensor(of2, bb2_s[:], of2, OP.subtract)

        # a2 + final matmul per block; block b lands on y partition b so the
        # final DMA is one multi-partition transfer
        a2_s = mlp_pool.tile([M2, sl], BF16, tag="a2")
        y_s = mlp_pool.tile([8, 512], F32, tag="y")
        pv8 = ps_a.tile([8, 512], F32, tag="p1")
        for b in range(nzb):
            a, e = 512 * b, min(sl, 512 * (b + 1))
            w = e - a
            nc.scalar.activation(a2_s[:, a:e], v2_s[:, a:e], ACTF.Relu,
                                 bias=of2, scale=sc2)
            # stationary slice b has w3 only in column b: block b accumulates
            # onto y row b (other rows += 0)
            nc.tensor.matmul(pv8[:, :w], mw3t_s[:, 8 * b:8 * b + 8],
                             a2_s[:, a:e], start=(b == 0),
                             stop=(b == nzb - 1))
        nc.vector.tensor_copy(y_s[:], pv8[:])
        nc.sync.dma_start(y_out[0:nzb, :], y_s[0:nzb, :])

    nc.compile()
    return nc


def _mw3_blocks(mlp_w3):
    m = np.zeros((M2, 64), dtype=np.float32)
    for b in range(8):
        m[:, 8 * b + b] = mlp_w3[0]
    return np.ascontiguousarray(m.astype(ml_dtypes.bfloat16))


_CACHE = {}


def kernel(**inputs) -> np.ndarray:
    x = np.asarray(inputs["x_set"], np.float32)[0]        # [16, T]
    x_scalar = np.asarray(inputs["x_scalar"], np.float32)  # [B, 8]
    lengths = np.asarray(inputs["sample_indices"])[0].astype(np.int64)
    b_total = x_scalar.shape[0]
    offsets = np.concatenate([[0], np.cumsum(lengths)[:-1]])

    groups, ng = _plan_groups(lengths, b_total)
    nchunk = ng // 2
    sl = nchunk * 16
    per_core = [
        _prep_core(x, x_scalar, lengths, offsets, groups[c], ng)
        for c in range(N_CORES)
    ]

    b1 = np.asarray(inputs["phi_b1"], np.float32)
    b2 = np.asarray(inputs["phi_b2"], np.float32)
    assert np.all(b1 == 0.0) and np.all(b2 == 0.0), \
        "nonzero phi bias path not implemented"
    w1s = np.zeros((2 * C_IN, 128), dtype=np.float32)
    w1s[0:C_IN, 0:F] = np.asarray(inputs["phi_w1"], np.float32).T
    w1s[C_IN:2 * C_IN, F:128] = np.asarray(inputs["phi_w1"], np.float32).T
    w2s = np.zeros((128, 128), dtype=np.float32)
    w2s[0:F, 0:F] = np.asarray(inputs["phi_w2"], np.float32).T
    w2s[F:128, F:128] = np.asarray(inputs["phi_w2"], np.float32).T
    mw1 = np.asarray(inputs["mlp_w1"], np.float32)        # [128, 72]
    w1a = np.zeros((128, M1), dtype=np.float32)
    w1a[0:F, :] = mw1[:, 0:F].T
    w1b = np.zeros((128, M1), dtype=np.float32)
    w1b[F:128, :] = mw1[:, 0:F].T
    w1x = np.ascontiguousarray(mw1[:, F:F + S_SCALAR].T)
    consts = {
        "w1s": np.ascontiguousarray(w1s.astype(ml_dtypes.bfloat16)),
        "w2s": np.ascontiguousarray(w2s.astype(ml_dtypes.bfloat16)),
        "identf": np.ascontiguousarray(np.eye(128, dtype=np.float32)),
        "w1a": np.ascontiguousarray(w1a.astype(ml_dtypes.bfloat16)),
        "w1b": np.ascontiguousarray(w1b.astype(ml_dtypes.bfloat16)),
        "w1x": np.ascontiguousarray(w1x.astype(ml_dtypes.bfloat16)),
        "mw2t": np.ascontiguousarray(
            np.asarray(inputs["mlp_w2"], np.float32).T.astype(ml_dtypes.bfloat16)),
        "mw3t": _mw3_blocks(np.asarray(inputs["mlp_w3"], np.float32)),
        "bn1_g": np.asarray(inputs["bn1_g"], np.float32).reshape(M1, 1),
        "bn1_b": np.asarray(inputs["bn1_b"], np.float32).reshape(M1, 1),
        "bn2_g": np.asarray(inputs["bn2_g"], np.float32).reshape(M2, 1),
        "bn2_b": np.asarray(inputs["bn2_b"], np.float32).reshape(M2, 1),
    }

    key = (ng, sl, b_total)
    if key not in _CACHE:
        _CACHE[key] = _build_nc(ng, sl, b_total)
    nc = _CACHE[key]

    in_maps = []
    for pc in per_core:
        m = {"xb": pc["xb"], "S": pc["S"], "xsT": pc["xsT"]}
        m.update(consts)
        in_maps.append(m)

    res = bass_utils.run_bass_kernel_spmd(
        nc, in_maps, core_ids=list(range(N_CORES)))

    b3 = float(np.asarray(inputs["mlp_b3"], np.float32).reshape(-1)[0])
    y = np.zeros((b_total, 1), dtype=np.float32)
    for c, pc in enumerate(per_core):
        u = res.results[c]["y"].reshape(-1)[:sl].astype(np.float64) + b3
        ys = 1.0 / (1.0 + np.exp(-u))
        se = pc["slot_events"]
        mask = se >= 0
        y[se[mask], 0] = ys[mask].astype(np.float32)
    return y
